# revision 2
# baseline (speedup 1.0000x reference)
"""Multi-head attention (B=16, GS=1024, E=768, H=12, D=64) on 8 trn2 NeuronCores.

Sharding: data-parallel over batch - 2 batches per core, no collectives.

Per-core design (per batch of S=1024 tokens):
  1. qkT = (x @ w_qk)^T -> [2E, S] (head-dim on partitions); q gets its bias
     (the k bias is softmax-invariant and dropped), v = x @ w_v in natural
     layout plus a ones column per head (v bias is folded into the proj bias
     on the host: b_eff = b_proj + b_v @ w_proj).
  2. heads in pairs (two 64-dim heads share the 128 PE rows via tile_position
     row groups): scoresT[ki,qi] matmuls, exp on ACT (scale=1/8 fused, no max
     subtraction - scores ~ N(0,1)), PV matmul with M=D+1 (ones column of v
     gives the softmax denominator row for free).
  3. normalization fully on-chip: reciprocal of the denominator rows, then a
     PE "broadcast" matmul (constant selector matrix x denom rows) expands
     them across partitions into PSUM; one DVE multiply applies them.
  4. proj: y = attnT^T @ w_proj + b_eff, accumulated per ec chunk (= head
     pair), so only the last accumulation step waits on the last pair's norm.

Weights stream per 128-column chunk in the exact order the first matmuls
consume them, so the PE starts ~3us into the kernel instead of waiting for
the full wqkv load. Batch b+1's prep fills batch b's head phase; batch b's
proj fills batch b+1's head phase.
"""

import numpy as np
from contextlib import ExitStack

import concourse.bass as bass
import concourse.mybir as mybir
import concourse.tile as tile
from concourse import bacc

F32 = mybir.dt.float32
BF16 = mybir.dt.bfloat16
AF = mybir.ActivationFunctionType
P = 128


def build_nc(BPC=2, S=1024, E=768, H=12, D=64, act_dtype=BF16):
    SCALE = D ** -0.5
    E3 = 3 * E
    EC = E // P              # emb chunks (6)
    SC = S // P              # seq chunks per batch (8)
    QT = 512                 # qi tile size
    NQT = S // QT            # qi tiles per batch (2)
    HPC = P // D             # heads per 128-chunk (pair size, 2)
    NPAIR = H // HPC         # 6
    T = BPC * S
    DV = D + 1               # v columns incl. ones
    NPLANE = (H + 3) // 4    # denominator planes (head -> partition 32*(h%4))

    nc = bacc.Bacc("TRN2", target_bir_lowering=False, debug=False)

    x_d = nc.dram_tensor("x_local", [E, T], act_dtype, kind="ExternalInput")
    wqkv_d = nc.dram_tensor("w_qkv", [E, E3], act_dtype, kind="ExternalInput")
    bqkv_d = nc.dram_tensor("b_qkv", [E3], F32, kind="ExternalInput")
    wproj_d = nc.dram_tensor("w_proj", [E, E], act_dtype, kind="ExternalInput")
    beff_d = nc.dram_tensor("b_eff", [E], F32, kind="ExternalInput")
    y_d = nc.dram_tensor("y_local", [T, E], F32, kind="ExternalOutput")

    def bcast_part(ap, n):
        return bass.AP(tensor=ap.tensor, offset=ap.offset, ap=[[0, n]] + list(ap.ap))

    # column-chunk order for the qk part of wqkv: q chunk then its k partner
    M_ORDER = []
    for i in range(EC):
        M_ORDER += [i, EC + i]

    with tile.TileContext(nc) as tc, ExitStack() as ctx:
        const = ctx.enter_context(tc.tile_pool(name="const", bufs=1))
        xtp = ctx.enter_context(tc.tile_pool(name="xtp", bufs=1))
        qkp = ctx.enter_context(tc.tile_pool(name="qkp", bufs=2))
        vp = ctx.enter_context(tc.tile_pool(name="vp", bufs=2))
        atp = ctx.enter_context(tc.tile_pool(name="atp", bufs=2))
        expp = ctx.enter_context(tc.tile_pool(name="expp", bufs=2))
        outp = ctx.enter_context(tc.tile_pool(name="outp", bufs=2))
        denp = ctx.enter_context(tc.tile_pool(name="denp", bufs=2))
        ps_sc = ctx.enter_context(tc.tile_pool(name="ps_sc", bufs=2, space="PSUM"))
        ps_pv = ctx.enter_context(tc.tile_pool(name="ps_pv", bufs=2, space="PSUM"))
        ps_pr = ctx.enter_context(tc.tile_pool(name="ps_pr", bufs=2, space="PSUM"))

        # ---------------- constants / weights ----------------
        wqkv_sb = const.tile([P, EC, E3], act_dtype, name="wqkv_sb")
        wproj_sb = const.tile([P, EC, E], act_dtype, name="wproj_sb")

        warm = const.tile([P, 1], F32)
        nc.vector.memset(warm, 0.0)
        nc.scalar.activation(warm, warm, AF.Exp, scale=1.0)

        # q bias per (partition, chunk)
        bq_sb = const.tile([P, EC], F32)
        with nc.allow_non_contiguous_dma(reason="tiny strided bias load"):
            nc.sync.dma_start(bq_sb, bqkv_d.ap()[0:E].rearrange("(c p) -> p c", p=P))
        # effective proj bias, broadcast across partitions
        beff_bc = const.tile([P, E], F32)
        nc.gpsimd.dma_start(beff_bc, bcast_part(beff_d.ap(), P))

        # selector matrices for the denominator broadcast matmul.
        # pair pr heads (2pr, 2pr+1) live at den partitions 32*(h%4):
        #   pr even -> partitions {0, 32}; pr odd -> {64, 96}
        sel = const.tile([P, 2, P], act_dtype, name="sel")
        nc.vector.memset(sel, 0.0)
        nc.vector.memset(sel[0:1, 0, 0:D], 1.0)
        nc.vector.memset(sel[32:33, 0, D:2 * D], 1.0)
        nc.vector.memset(sel[64:65, 1, 0:D], 1.0)
        nc.vector.memset(sel[96:97, 1, D:2 * D], 1.0)

        wq_view = wqkv_d.ap().rearrange("(ec p) c -> p ec c", p=P)

        def wq_load(m):
            nc.sync.dma_start(wqkv_sb[:, :, m * P:(m + 1) * P],
                              wq_view[:, :, m * P:(m + 1) * P])

        def wv_load(k):
            c0 = 2 * E + k * (E // 2)
            nc.sync.dma_start(wqkv_sb[:, :, c0:c0 + E // 2],
                              wq_view[:, :, c0:c0 + E // 2])

        def wp_load(ec):
            nc.sync.dma_start(wproj_sb[:, ec, :], wproj_d[ec * P:(ec + 1) * P, :])

        states = {}

        # ---------------- unit builders ----------------
        def u_alloc(b):
            st = states.setdefault(b, {})

            def u():
                st["xT"] = [xtp.tile([P, EC, QT], act_dtype, name=f"xT{b}_{qi}",
                                     tag=f"xT{qi}") for qi in range(NQT)]
                st["qkT"] = qkp.tile([P, 2 * EC, S], act_dtype, name=f"qkT{b}",
                                     tag="qkT")
                st["v"] = vp.tile([P, SC, H, DV], act_dtype, name=f"v{b}", tag="v")
                st["attnT"] = atp.tile([P, EC, S], act_dtype, name=f"attnT{b}",
                                       tag="attnT")
                st["den"] = denp.tile([P, NPLANE, S], act_dtype, name=f"den{b}",
                                      tag="den")
                nc.vector.memset(st["v"][:, :, :, D:DV], 1.0)
                nc.gpsimd.memset(st["den"], 1.0)
            return u

        def u_xdma(b, qi):
            def u():
                st = states[b]
                xt_view = x_d.ap().rearrange("(ec p) t -> p ec t", p=P)
                nc.sync.dma_start(
                    st["xT"][qi][:, :, :],
                    xt_view[:, :, b * S + qi * QT: b * S + (qi + 1) * QT])
            return u

        def u_qk(b, m, qi):
            def u():
                st = states[b]
                pt = ps_pr.tile([P, 512], F32, tag="pr", name=f"qk{b}_{m}_{qi}")
                for ec in range(EC):
                    nc.tensor.matmul(
                        pt[:, 0:QT],
                        wqkv_sb[:, ec, m * P:(m + 1) * P],
                        st["xT"][qi][:, ec, :],
                        start=(ec == 0), stop=(ec == EC - 1),
                    )
                dst = st["qkT"][:, m, qi * QT:(qi + 1) * QT]
                if m < EC:   # q chunk: add bias
                    nc.vector.tensor_scalar_add(dst, pt[:, 0:QT], bq_sb[:, m:m + 1])
                else:        # k chunk: bias dropped (softmax-invariant)
                    nc.vector.tensor_copy(dst, pt[:, 0:QT])
            return u

        V_NTS = [(0, 512), (512, 256)]

        def u_v(b, si, k):
            def u():
                st = states[b]
                nt, n_sl = V_NTS[k]
                pt = ps_pr.tile([P, 512], F32, tag="pr", name=f"v{b}_{si}_{k}")
                qi, so = divmod(si * P, QT)
                for ec in range(EC):
                    nc.tensor.matmul(
                        pt[:, 0:n_sl],
                        st["xT"][qi][:, ec, so:so + P],
                        wqkv_sb[:, ec, 2 * E + nt: 2 * E + nt + n_sl],
                        start=(ec == 0), stop=(ec == EC - 1),
                    )
                nc.vector.tensor_copy(
                    st["v"][:, si, nt // D: (nt + n_sl) // D, 0:D],
                    pt[:, 0:n_sl].rearrange("p (h d) -> p h d", d=D))
            return u

        def u_sc_exp(b, pr, qi, kc, ep):
            def u():
                st = states[b]
                qkT = st["qkT"]
                ps = ps_sc.tile([P, HPC, 512], F32, tag="sc")
                for j in range(HPC):
                    po = D * j
                    nc.tensor.matmul(
                        ps[:, j, 0:QT],
                        qkT[po:po + D, EC + pr, kc * P:(kc + 1) * P],
                        qkT[po:po + D, pr, qi * QT:(qi + 1) * QT],
                        start=True, stop=True,
                        tile_position=(po, 0),
                    )
                nc.scalar.activation(ep[:, kc, :, :], ps[:, :, 0:QT],
                                     AF.Exp, scale=SCALE)
            return u

        def u_pv(b, pr, qi, kc, pvt, ep):
            def u():
                st = states[b]
                for j in range(HPC):
                    h = pr * HPC + j
                    nc.tensor.matmul(
                        pvt[j][0:DV, 0:QT],
                        st["v"][:, kc, h, :],
                        ep[:, kc, j, :],
                        start=(kc == 0), stop=(kc == SC - 1),
                    )
            return u

        def u_fin(b, pr, qi, pvt):
            def u():
                st = states[b]
                attnT, den = st["attnT"], st["den"]
                for j in range(HPC):
                    h = pr * HPC + j
                    po = D * j
                    dr = 32 * (h % 4)
                    nc.vector.tensor_copy(
                        den[dr:dr + 1, h // 4, qi * QT:(qi + 1) * QT],
                        pvt[j][D:DV, 0:QT])
                    nc.vector.tensor_copy(
                        attnT[po:po + D, pr, qi * QT:(qi + 1) * QT],
                        pvt[j][0:D, 0:QT])
            return u

        def u_norm(b, pr, qi):
            def u():
                st = states[b]
                attnT, den = st["attnT"], st["den"]
                with nc.allow_low_precision(reason="softmax denom in act dtype"):
                    for j in range(HPC):
                        h = pr * HPC + j
                        dr = 32 * (h % 4)
                        d_ap = den[dr:dr + 1, h // 4, qi * QT:(qi + 1) * QT]
                        nc.vector.reciprocal(d_ap, d_ap)
                rb = ps_pr.tile([P, 512], F32, tag="pr", name=f"rb{b}_{pr}_{qi}")
                nc.tensor.matmul(
                    rb[:, 0:QT],
                    sel[:, pr % 2, :],
                    den[:, pr // 2, qi * QT:(qi + 1) * QT],
                    start=True, stop=True,
                )
                a_ap = attnT[:, pr, qi * QT:(qi + 1) * QT]
                nc.vector.tensor_mul(a_ap, a_ap, rb[:, 0:QT])
            return u

        def u_proj(b, si, k):
            def u():
                st = states[b]
                nt, n_sl = V_NTS[k]
                pt = ps_pr.tile([P, 512], F32, tag="pr", name=f"pj{b}_{si}_{k}")
                for ec in range(EC):
                    nc.tensor.matmul(
                        pt[:, 0:n_sl],
                        st["attnT"][:, ec, si * P:(si + 1) * P],
                        wproj_sb[:, ec, nt:nt + n_sl],
                        start=(ec == 0), stop=(ec == EC - 1),
                    )
                yt = outp.tile([P, 512], F32, tag="y", name=f"y{b}_{si}_{k}")
                nc.vector.tensor_add(yt[:, 0:n_sl], pt[:, 0:n_sl],
                                     beff_bc[:, nt:nt + n_sl])
                nc.sync.dma_start(
                    y_d[b * S + si * P: b * S + (si + 1) * P, nt:nt + n_sl],
                    yt[:, 0:n_sl])
            return u

        def prep_units(b):
            us = []
            for m in M_ORDER:
                for qi in range(NQT):
                    us.append(u_qk(b, m, qi))
            for si in range(SC):
                for k in range(2):
                    us.append(u_v(b, si, k))
            return us

        def head_group(b, pr, qi, fillers):
            """Emit one (pair, qi) head group, popping filler units inside."""
            st = states[b]
            ep = expp.tile([P, SC, HPC, QT], act_dtype, tag="exp",
                           name=f"ep{b}_{pr}_{qi}")
            for kc in range(SC):
                u_sc_exp(b, pr, qi, kc, ep)()
            if fillers:
                fillers.pop(0)()
            pvt = [ps_pv.tile([P, 512], F32, tag="pv", name=f"pv{b}_{pr}_{qi}_{j}")
                   for j in range(HPC)]
            for kc in range(SC):
                u_pv(b, pr, qi, kc, pvt, ep)()
                if kc % 3 == 2 and fillers:
                    fillers.pop(0)()
            u_fin(b, pr, qi, pvt)()
            u_norm(b, pr, qi)()
            if fillers:
                fillers.pop(0)()

        # ---------------- emission schedule ----------------
        u_alloc(0)()
        u_xdma(0, 0)()
        wq_load(M_ORDER[0])
        u_xdma(0, 1)()
        for m in M_ORDER[1:]:
            wq_load(m)
        wv_load(0)
        wv_load(1)
        for ec in range(EC):
            wp_load(ec)
        for u in prep_units(0):
            u()

        # batch 0 heads, filled with batch 1 prep
        u_alloc(1)()
        fillers = [u_xdma(1, 0), u_xdma(1, 1)] + prep_units(1)
        for pr in range(NPAIR):
            for qi in range(NQT):
                head_group(0, pr, qi, fillers)
        for u in fillers:
            u()

        # batch 1 heads, filled with batch 0 proj
        fillers = []
        for si in range(SC):
            for k in range(2):
                fillers.append(u_proj(0, si, k))
        for pr in range(NPAIR):
            for qi in range(NQT):
                head_group(1, pr, qi, fillers)
        for u in fillers:
            u()

        # batch 1 proj tail
        for si in range(SC):
            for k in range(2):
                u_proj(1, si, k)()

    nc.compile()
    return nc


_NC_CACHE = {}


def _get_nc():
    if "nc" not in _NC_CACHE:
        _NC_CACHE["nc"] = build_nc()
    return _NC_CACHE["nc"]


B, GS, E_FULL = 16, 1024, 768
N_CORES = 8
BPC_FULL = B // N_CORES


def make_in_maps(x, w_qkv, b_qkv, w_proj, b_proj):
    import ml_dtypes
    bf = ml_dtypes.bfloat16
    x = np.asarray(x, dtype=np.float32).astype(bf)  # [B, GS, E]
    w_qkv_f = np.asarray(w_qkv, dtype=np.float32)
    b_qkv_f = np.asarray(b_qkv, dtype=np.float32)
    w_proj_f = np.asarray(w_proj, dtype=np.float32)
    b_proj_f = np.asarray(b_proj, dtype=np.float32)
    # fold the v bias through the projection: out = (attn + b_v) @ w_proj + b
    b_eff = (b_proj_f.astype(np.float64)
             + b_qkv_f[2 * E_FULL:].astype(np.float64)
             @ w_proj_f.astype(np.float64)).astype(np.float32)
    w_qkv_b = np.ascontiguousarray(w_qkv_f.astype(bf))
    w_proj_b = np.ascontiguousarray(w_proj_f.astype(bf))
    b_qkv_c = np.ascontiguousarray(b_qkv_f)
    in_maps = []
    for i in range(N_CORES):
        in_maps.append({
            "x_local": np.ascontiguousarray(
                x[i * BPC_FULL:(i + 1) * BPC_FULL].reshape(BPC_FULL * GS, E_FULL).T),
            "w_qkv": w_qkv_b, "b_qkv": b_qkv_c,
            "w_proj": w_proj_b, "b_eff": b_eff,
        })
    return in_maps


def gather_out(results):
    return np.concatenate(
        [r["y_local"].reshape(BPC_FULL, GS, E_FULL) for r in results],
        axis=0).astype(np.float32)


def kernel(x, w_qkv, b_qkv, w_proj, b_proj):
    from concourse.bass_utils import run_bass_kernel_spmd

    nc = _get_nc()
    in_maps = make_in_maps(x, w_qkv, b_qkv, w_proj, b_proj)
    res = run_bass_kernel_spmd(nc, in_maps, core_ids=list(range(N_CORES)))
    return gather_out(res.results)
